# revision 1
# baseline (speedup 1.0000x reference)
"""Trainium2 Bass kernel for nn_ExpertsFeedForward (MoE expert-choice routing).

Sharding: owner-parallel over tokens. Each of the 8 cores owns a 2048-token
slice of the flattened [16384, 1024] input and produces that slice of the
output. FF expert weights are replicated (streamed from HBM, bf16); each core
computes the shared expert for its tokens plus every FF expert's contribution
to its tokens (dma_gather -> FFN -> score scale -> CCE dma_scatter_add), with
the constant 'jump' expert and all biases folded in as rank-1 matmuls.
"""

import numpy as np
import ml_dtypes

import concourse.bass as bass
import concourse.mybir as mybir
import concourse.bacc as bacc
import concourse.tile as tile
from concourse.bass_utils import run_bass_kernel_spmd
from concourse.library_config import mlp as mlp_lib

BF16 = mybir.dt.bfloat16
F32 = mybir.dt.float32
I16 = mybir.dt.int16
U32 = mybir.dt.uint32

NC = 8            # cores
D = 1024          # d_model
H = 4096          # d_ff
FF = 7            # matmul experts
NTOK = 16384      # total tokens
TOK = NTOK // NC  # tokens per core (2048)
CAP = 1638        # expert capacity (global)
BUCKET = 256      # per-(core, expert) selected-token capacity (padded)
KD = D // 128     # 8  contraction tiles over d
MH = H // 128     # 32 tiles over h
ND2 = 2           # 512-wide n chunks over D
CH = 256          # shared-FFN token chunk
NCH = TOK // CH   # 8 chunks
TT = TOK // 128   # 16 row tiles
GELU = mybir.ActivationFunctionType.Gelu_apprx_tanh


def build_program(parts=("experts", "shared", "final"), gather=True,
                  scatter=True, reps=1):
    nc = bacc.Bacc("TRN2", target_bir_lowering=False, debug=False, num_devices=NC)

    # ---- per-core external inputs ----
    xT = nc.dram_tensor("xT", [KD, 128, NCH, CH], BF16, kind="ExternalInput")
    xrows = nc.dram_tensor("xrows", [TOK, D], BF16, kind="ExternalInput")
    wk = nc.dram_tensor("wk", [FF, KD, MH, 128, 128], BF16, kind="ExternalInput")
    wv = nc.dram_tensor("wv", [FF, MH, ND2, 128, 512], BF16, kind="ExternalInput")
    swk = nc.dram_tensor("swk", [KD, MH, 128, 128], BF16, kind="ExternalInput")
    swv = nc.dram_tensor("swv", [MH, ND2, 128, 512], BF16, kind="ExternalInput")
    bk = nc.dram_tensor("bk", [FF, MH, 128], F32, kind="ExternalInput")
    sbk = nc.dram_tensor("sbk", [MH, 128], F32, kind="ExternalInput")
    bvr = nc.dram_tensor("bvr", [FF, 1, D], BF16, kind="ExternalInput")
    sbvr = nc.dram_tensor("sbvr", [1, D], BF16, kind="ExternalInput")
    jumpv = nc.dram_tensor("jumpv", [1, D], BF16, kind="ExternalInput")
    onesr = nc.dram_tensor("onesr", [1, 128], BF16, kind="ExternalInput")
    idxg = nc.dram_tensor("idxg", [FF, 128, BUCKET // 16], I16, kind="ExternalInput")
    idxs = nc.dram_tensor("idxs", [FF, 128, BUCKET // 16], I16, kind="ExternalInput")
    scores = nc.dram_tensor("scores", [FF, BUCKET // 128, 128], F32,
                            kind="ExternalInput")
    ms7 = nc.dram_tensor("ms7", [1, TOK], BF16, kind="ExternalInput")

    out = nc.dram_tensor("out", [TOK, D], F32, kind="ExternalOutput")

    # ---- internal DRAM scratch ----
    eout = nc.dram_tensor("eout", [TOK + 128, D], BF16)  # + dummy pad rows
    sout = nc.dram_tensor("sout", [TOK, D], F32)    # shared+jump dense

    with tile.TileContext(nc) as tc:
        with (
            tc.tile_pool(name="const", bufs=1) as cpool,
            tc.tile_pool(name="psum1", bufs=4, space="PSUM") as pp1,
            tc.tile_pool(name="psum2", bufs=4, space="PSUM") as pp2,
        ):
            nc.gpsimd.load_library(mlp_lib)
            _eng_rr = [nc.sync, nc.scalar]
            def rr_dma(i, **kw):
                _eng_rr[i % len(_eng_rr)].dma_start(**kw)

            def gelu_tanh(out_ap, ps_ap, bias_ap, tmp_pool, n, tag):
                """out = gelu_tanh(ps + bias); ps is PSUM f32 [128, n]."""
                xb = tmp_pool.tile([128, n], F32, tag=tag + "x")
                s = tmp_pool.tile([128, n], F32, tag=tag + "s")
                nc.vector.tensor_scalar_add(xb[:], ps_ap, bias_ap)
                nc.vector.tensor_tensor(s[:], xb[:], xb[:], op=mybir.AluOpType.mult)
                nc.vector.tensor_scalar(s[:], s[:], 0.044715, 1.0,
                                        op0=mybir.AluOpType.mult,
                                        op1=mybir.AluOpType.add)
                nc.vector.tensor_tensor(s[:], s[:], xb[:], op=mybir.AluOpType.mult)
                nc.scalar.activation(s[:], s[:], mybir.ActivationFunctionType.Tanh,
                                     scale=0.7978845608028654)
                nc.vector.tensor_scalar(s[:], s[:], 0.5, 0.5,
                                        op0=mybir.AluOpType.mult,
                                        op1=mybir.AluOpType.add)
                nc.vector.tensor_tensor(out_ap, s[:], xb[:],
                                        op=mybir.AluOpType.mult)

            # --- constants resident in SBUF for the whole kernel ---
            ones_sb = cpool.tile([1, 128], BF16, tag="ones")
            nc.sync.dma_start(out=ones_sb[:], in_=onesr[:])
            jump_sb = cpool.tile([1, D], BF16, tag="jump")
            nc.sync.dma_start(out=jump_sb[:], in_=jumpv[:])
            sbv_sb = cpool.tile([1, D], BF16, tag="sbv")
            nc.sync.dma_start(out=sbv_sb[:], in_=sbvr[:])
            bv_sb = cpool.tile([1, FF * D], BF16, tag="bv")
            nc.sync.dma_start(out=bv_sb[:], in_=bvr[:].rearrange("e o d -> o (e d)"))
            ms7_sb = cpool.tile([1, TOK], BF16, tag="ms7")
            nc.sync.dma_start(out=ms7_sb[:], in_=ms7[:])
            zero_sb = cpool.tile([128, D], BF16, tag="zero")
            nc.vector.memset(zero_sb[:], 0.0)
            for t in range(TT + 1):
                nc.sync.dma_start(out=eout[t * 128:(t + 1) * 128, :], in_=zero_sb[:])

            for _rep in range(reps):
                # ================= phase 1: FF experts =================
                with (
                    tc.tile_pool(name="ep_io", bufs=2) as eio,
                    tc.tile_pool(name="ep_w", bufs=3) as ewp,
                    tc.tile_pool(name="ep_acts", bufs=1) as eap,
                ):
                    for e in (range(FF) if "experts" in parts else []):
                        idxg_sb = eio.tile([128, BUCKET // 16], I16, tag="idxg")
                        nc.sync.dma_start(out=idxg_sb[:], in_=idxg[e])
                        idxs_sb = eio.tile([128, BUCKET // 16], I16, tag="idxs")
                        nc.sync.dma_start(out=idxs_sb[:], in_=idxs[e])
                        xs = eap.tile([128, KD, BUCKET], BF16, tag="xs")
                        if gather:
                            nc.gpsimd.dma_gather(
                                xs[:], xrows[:], idxg_sb[:], BUCKET, BUCKET, D,
                                transpose=True,
                            )
                        else:
                            nc.vector.memset(xs[:], 0.25)
                        # expert weights resident, one batched DMA each
                        wv_sb = eap.tile([128, MH, ND2, 512], BF16, tag="wv_e")
                        nc.sync.dma_start(
                            out=wv_sb[:], in_=wv[e].rearrange("m n p h -> p m n h"))
                        wk_sb = eap.tile([128, KD, MH, 128], BF16, tag="wk_e")
                        nc.scalar.dma_start(
                            out=wk_sb[:], in_=wk[e].rearrange("k m p h -> p k m h"))
                        bk_sb = eio.tile([128, MH], F32, tag="bk")
                        nc.sync.dma_start(out=bk_sb[:], in_=bk[e].rearrange("a p -> p a"))
                        sc_sb = eio.tile([128, BUCKET // 128], F32, tag="sc")
                        nc.sync.dma_start(
                            out=sc_sb[:], in_=scores[e].rearrange("a p -> p a"))

                        hT = eap.tile([128, MH, BUCKET], BF16, tag="hT_e")
                        for m in range(MH):
                            ps = pp1.tile([128, BUCKET], F32, tag="ps1")
                            for k in range(KD):
                                nc.tensor.matmul(ps[:], wk_sb[:, k, m, :], xs[:, k, :],
                                                 start=(k == 0), stop=(k == KD - 1))
                            gelu_tanh(hT[:, m, :], ps[:], bk_sb[:, m:m + 1],
                                      eio, BUCKET, "ge")
                        ysb = eap.tile([128, BUCKET // 128, D], BF16, tag="ysb")
                        for tt in range(BUCKET // 128):
                            for n in range(ND2):
                                ps2 = pp2.tile([128, 512], F32, tag="ps2")
                                for m in range(MH):
                                    nc.tensor.matmul(
                                        ps2[:], hT[:, m, tt * 128:(tt + 1) * 128],
                                        wv_sb[:, m, n, :], start=(m == 0), stop=False)
                                nc.tensor.matmul(
                                    ps2[:], ones_sb[:],
                                    bv_sb[:, e * D + n * 512:e * D + (n + 1) * 512],
                                    start=False, stop=True)
                                nc.vector.tensor_scalar_mul(
                                    ysb[:, tt, n * 512:(n + 1) * 512], ps2[:],
                                    sc_sb[:, tt:tt + 1])
                        if scatter:
                            nc.gpsimd.dma_scatter_add(
                                eout[:], ysb[:], idxs_sb[:], BUCKET, BUCKET, D,
                                queue_num=0)

                # ================= phase 2: shared expert =================
                with (
                    tc.tile_pool(name="sp_io", bufs=2) as sio,
                    tc.tile_pool(name="sp_w", bufs=1) as swp,
                    tc.tile_pool(name="sp_acts", bufs=2) as sap,
                ):
                    swk_sb = swp.tile([128, KD, MH, 128], BF16, tag="swk")
                    nc.sync.dma_start(
                        out=swk_sb[:], in_=swk[:].rearrange("k m p h -> p k m h"))
                    swv_sb = swp.tile([128, MH, ND2, 512], BF16, tag="swv")
                    nc.scalar.dma_start(
                        out=swv_sb[:], in_=swv[:].rearrange("m n p h -> p m n h"))
                    sbk_sb = swp.tile([128, MH], F32, tag="sbk")
                    nc.sync.dma_start(out=sbk_sb[:], in_=sbk[:].rearrange("a p -> p a"))

                    for c in (range(NCH) if "shared" in parts else []):
                        xc = sio.tile([128, KD, CH], BF16, tag="xc")
                        nc.sync.dma_start(
                            out=xc[:], in_=xT[:, :, c, :].rearrange("k p j -> p k j"))
                        hT = sap.tile([128, MH, CH], BF16, tag="hT_s")
                        for m in range(MH):
                            ps = pp1.tile([128, CH], F32, tag="ps1")
                            for k in range(KD):
                                nc.tensor.matmul(ps[:], swk_sb[:, k, m, :], xc[:, k, :],
                                                 start=(k == 0), stop=(k == KD - 1))
                            gelu_tanh(hT[:, m, :], ps[:], sbk_sb[:, m:m + 1],
                                      sio, CH, "gs")
                        for tt in range(CH // 128):
                            g0 = c * CH + tt * 128
                            so = sap.tile([128, D], F32, tag="so")
                            for n in range(ND2):
                                ps2 = pp2.tile([128, 512], F32, tag="ps2")
                                for m in range(MH):
                                    nc.tensor.matmul(
                                        ps2[:], hT[:, m, tt * 128:(tt + 1) * 128],
                                        swv_sb[:, m, n, :], start=(m == 0), stop=False)
                                nc.tensor.matmul(
                                    ps2[:], ones_sb[:], sbv_sb[:, n * 512:(n + 1) * 512],
                                    start=False, stop=False)
                                nc.tensor.matmul(
                                    ps2[:], ms7_sb[:, g0:g0 + 128],
                                    jump_sb[:, n * 512:(n + 1) * 512],
                                    start=False, stop=True)
                                nc.vector.tensor_copy(so[:, n * 512:(n + 1) * 512], ps2[:])
                            nc.sync.dma_start(out=sout[g0:g0 + 128, :], in_=so[:])

                # ================= phase 3: combine =================
                with tc.tile_pool(name="fp", bufs=3) as fp:
                    for t in (range(TT) if "final" in parts else []):
                        a = fp.tile([128, D], F32, tag="fa")
                        b = fp.tile([128, D], BF16, tag="fb")
                        bf = fp.tile([128, D], F32, tag="fbf")
                        nc.sync.dma_start(out=a[:], in_=sout[t * 128:(t + 1) * 128, :])
                        nc.sync.dma_start(out=b[:], in_=eout[t * 128:(t + 1) * 128, :])
                        nc.vector.tensor_copy(bf[:], b[:])
                        nc.vector.tensor_add(a[:], a[:], bf[:])
                        nc.sync.dma_start(out=out[t * 128:(t + 1) * 128, :], in_=a[:])

    nc.compile()
    return nc


def _bf(a):
    return np.ascontiguousarray(a.astype(ml_dtypes.bfloat16))


def host_route(x_flat, gate_W, gate_b, temperature):
    """Replicates the reference router + expert-choice top-k in numpy."""
    logits = x_flat.astype(np.float32) @ gate_W + gate_b
    t = max(float(np.asarray(temperature).reshape(-1)[0]), 0.1)
    z = logits / t
    z = z - z.max(axis=1, keepdims=True)
    p = np.exp(z)
    p = p / p.sum(axis=1, keepdims=True)
    order = np.argsort(-p, axis=0, kind="stable")
    sel = order[:CAP]  # [CAP, 8]
    return p, sel


def prepare_in_maps(inputs):
    x = np.asarray(inputs["x"], dtype=np.float32).reshape(NTOK, D)
    p, sel = host_route(
        x, np.asarray(inputs["gate_W"], np.float32),
        np.asarray(inputs["gate_b"], np.float32),
        np.asarray(inputs["temperature"], np.float32),
    )

    wk_t = _bf(np.asarray(inputs["Wk"], np.float32)
               .reshape(FF, KD, 128, MH, 128).transpose(0, 1, 3, 2, 4))
    wv_t = _bf(np.asarray(inputs["Wv"], np.float32)
               .reshape(FF, MH, 128, ND2, 512).transpose(0, 1, 3, 2, 4))
    swk_t = _bf(np.asarray(inputs["sWk"], np.float32)
                .reshape(KD, 128, MH, 128).transpose(0, 2, 1, 3))
    swv_t = _bf(np.asarray(inputs["sWv"], np.float32)
                .reshape(MH, 128, ND2, 512).transpose(0, 2, 1, 3))
    bk_t = np.ascontiguousarray(
        np.asarray(inputs["bk"], np.float32).reshape(FF, MH, 128))
    sbk_t = np.ascontiguousarray(
        np.asarray(inputs["sbk"], np.float32).reshape(MH, 128))
    bvr = _bf(np.asarray(inputs["bv"], np.float32).reshape(FF, 1, D))
    sbvr = _bf(np.asarray(inputs["sbv"], np.float32).reshape(1, D))
    jumpv = _bf(np.asarray(inputs["jump"], np.float32).reshape(1, D))
    onesr = _bf(np.ones((1, 128), np.float32))

    in_maps = []
    for c in range(NC):
        lo, hi = c * TOK, (c + 1) * TOK
        xs = x[lo:hi]
        xT_c = _bf(xs.T.reshape(KD, 128, NCH, CH))
        idxg_c = np.zeros((FF, BUCKET), np.int16)
        idxs_c = np.zeros((FF, BUCKET), np.int16)
        sc_c = np.zeros((FF, BUCKET), np.float32)
        for e in range(FF):
            g = np.sort(sel[:, e][(sel[:, e] >= lo) & (sel[:, e] < hi)])
            n = len(g)
            assert 0 < n <= BUCKET, f"bucket count {n} out of range"
            idxg_c[e, :n] = (g - lo).astype(np.int16)
            idxs_c[e, :n] = (g - lo).astype(np.int16)
            idxs_c[e, n:] = TOK + np.arange(BUCKET - n, dtype=np.int16) % 128
            sc_c[e, :n] = p[g, e]
        # wrap indices [r] -> [r%16, r//16], tiled to 128 partitions
        def wrap(a):
            w = np.ascontiguousarray(a.reshape(FF, BUCKET // 16, 16).transpose(0, 2, 1))
            return np.tile(w, (1, 8, 1))
        idxg_w = wrap(idxg_c)
        idxs_w = wrap(idxs_c)
        sc_pt = np.ascontiguousarray(sc_c.reshape(FF, BUCKET // 128, 128))
        m7 = sel[:, FF][(sel[:, FF] >= lo) & (sel[:, FF] < hi)]
        ms7_c = np.zeros(TOK, np.float32)
        ms7_c[m7 - lo] = p[m7, FF]
        in_maps.append({
            "xT": xT_c,
            "xrows": _bf(xs),
            "wk": wk_t, "wv": wv_t, "swk": swk_t, "swv": swv_t,
            "bk": bk_t, "sbk": sbk_t, "bvr": bvr, "sbvr": sbvr,
            "jumpv": jumpv, "onesr": onesr,
            "idxg": idxg_w, "idxs": idxs_w, "scores": sc_pt,
            "ms7": _bf(ms7_c.reshape(1, TOK)),
        })
    return in_maps


_CACHED = None


def kernel(**inputs):
    global _CACHED
    if _CACHED is None:
        _CACHED = build_program()
    nc = _CACHED
    in_maps = prepare_in_maps(inputs)
    res = run_bass_kernel_spmd(nc, in_maps, list(range(NC)))
    out = np.concatenate([res.results[c]["out"] for c in range(NC)], axis=0)
    return out.reshape(8, 2048, 1024).astype(np.float32)


if __name__ == "__main__":
    d = np.load("/root/problem/ref_inputs.npz")
    exp = np.load("/root/problem/ref_out.npy")
    got = kernel(**{k: d[k] for k in d.files})
    err = np.abs(got - exp)
    print("absmax rel:", err.max() / np.abs(exp).max())
    print("rms rel:", np.sqrt((err ** 2).mean()) / exp.std())



# revision 4
# speedup vs baseline: 1.7536x; 1.7536x over previous
"""Trainium2 Bass kernel for nn_ExpertsFeedForward (MoE expert-choice routing).

Sharding: expert-parallel with host-side token dispatch. The router
(softmax + expert-choice top-k) runs on host, as does the gather/scatter-add
"all-to-all" and all rank-1 epilogues (bv/sbv biases, jump expert, score
scaling). Each core runs two dense FFN blocks over pre-gathered tokens:

  block A (1664 tok): cores 0-6 -> that core's FF expert; core 7 -> shared
  block B (1920 tok): a 1920-token slice of the shared expert's tokens

so per-core compute is a balanced ~3.5k token-passes of gelu-FFN. All device
DMAs are large contiguous transfers of host-prelaid tensors (no gpsimd
gather/scatter, no transposing DMA), gelu runs on the Activation engine with
the bk bias fused, and the two GEMMs per chunk keep PE busy continuously.
"""

import numpy as np
import ml_dtypes

import concourse.bass as bass
import concourse.mybir as mybir
import concourse.bacc as bacc
import concourse.tile as tile
from concourse.bass_utils import run_bass_kernel_spmd

BF16 = mybir.dt.bfloat16
F32 = mybir.dt.float32

NC = 8            # cores
B, S = 8, 2048
D = 1024          # d_model
H = 4096          # d_ff
FF = 7            # matmul experts
E = 8             # router experts (7 FF + jump)
NTOK = 16384      # total tokens
CAP = 1638        # expert capacity
KD = D // 128     # 8 contraction tiles over d
MH = H // 128     # 32 tiles over h
MH2 = MH // 2     # 16 (half-split for DMA descriptor sizing)
ND2 = 2           # 512-wide n chunks over D
NTA = 1664        # block-A tokens per core (13*128 >= CAP)
NTB = 1920        # block-B (shared) tokens per core (15*128)
NT = NTA + NTB    # 3584 tokens per core
GELU = mybir.ActivationFunctionType.Gelu_apprx_tanh


def _chunks(base, total):
    """Split [base, base+total) into 256-token chunks (+128 tail)."""
    out = []
    t = 0
    while t < total:
        cw = min(256, total - t)
        out.append((base + t, cw))
        t += cw
    return out


def build_program(reps=1):
    nc = bacc.Bacc("TRN2", target_bir_lowering=False, debug=False, num_devices=NC)

    # ---- per-core external inputs (host-prelaid, all contiguous) ----
    xT = nc.dram_tensor("xT", [128, KD, NT], BF16, kind="ExternalInput")
    wks = nc.dram_tensor("wks", [2, 2, 128, KD, MH2, 128], BF16, kind="ExternalInput")
    wvs = nc.dram_tensor("wvs", [2, 2, 128, MH2, ND2, 512], BF16, kind="ExternalInput")
    bks = nc.dram_tensor("bks", [2, 128, MH], F32, kind="ExternalInput")
    out = nc.dram_tensor("out", [NT, D], BF16, kind="ExternalOutput")

    with tile.TileContext(nc) as tc:
        with (
            tc.tile_pool(name="w", bufs=1) as wp,
            tc.tile_pool(name="io", bufs=2) as iop,
            tc.tile_pool(name="act", bufs=2) as ap_,
            tc.tile_pool(name="o", bufs=3) as op_,
            tc.tile_pool(name="ps1", bufs=2, space="PSUM") as pp1,
            tc.tile_pool(name="ps2", bufs=4, space="PSUM") as pp2,
        ):
            for _rep in range(reps):
                for blk in range(2):
                    chunks = _chunks(0 if blk == 0 else NTA,
                                     NTA if blk == 0 else NTB)
                    wk_sb = wp.tile([128, 2, KD, MH2, 128], BF16, tag="wk")
                    nc.sync.dma_start(out=wk_sb[:, 0], in_=wks[blk, 0])
                    bk_sb = iop.tile([128, MH], F32, tag="bk")
                    nc.sync.dma_start(out=bk_sb[:], in_=bks[blk])
                    # first x chunk ahead of the remaining weight halves so
                    # GEMM1 can start as soon as wk half 0 lands
                    t0, cw = chunks[0]
                    xc0 = iop.tile([128, KD, 256], BF16, tag="xc")
                    nc.sync.dma_start(out=xc0[:, :, :cw], in_=xT[:, :, t0:t0 + cw])
                    nc.sync.dma_start(out=wk_sb[:, 1], in_=wks[blk, 1])
                    wv_sb = wp.tile([128, 2, MH2, ND2, 512], BF16, tag="wv")
                    for h in range(2):
                        nc.scalar.dma_start(out=wv_sb[:, h], in_=wvs[blk, h])

                    for ci, (t0, cw) in enumerate(chunks):
                        if ci == 0:
                            xc = xc0
                        else:
                            xc = iop.tile([128, KD, 256], BF16, tag="xc")
                            nc.sync.dma_start(out=xc[:, :, :cw],
                                              in_=xT[:, :, t0:t0 + cw])
                        hT = ap_.tile([128, MH, 256], BF16, tag="hT")
                        for m in range(MH):
                            ps1 = pp1.tile([128, 256], F32, tag="ps1")
                            for k in range(KD):
                                nc.tensor.matmul(
                                    ps1[:, :cw], wk_sb[:, m // MH2, k, m % MH2, :],
                                    xc[:, k, :cw], start=(k == 0), stop=(k == KD - 1))
                            nc.scalar.activation(hT[:, m, :cw], ps1[:, :cw], GELU,
                                                 bias=bk_sb[:, m:m + 1])
                        for tt in range(cw // 128):
                            o_sb = op_.tile([128, D], BF16, tag="o")
                            for n in range(ND2):
                                ps2 = pp2.tile([128, 512], F32, tag="ps2")
                                for m in range(MH):
                                    nc.tensor.matmul(
                                        ps2[:], hT[:, m, tt * 128:(tt + 1) * 128],
                                        wv_sb[:, m // MH2, m % MH2, n, :],
                                        start=(m == 0), stop=(m == MH - 1))
                                nc.vector.tensor_copy(o_sb[:, n * 512:(n + 1) * 512],
                                                      ps2[:])
                            g0 = t0 + tt * 128
                            nc.sync.dma_start(out=out[g0:g0 + 128, :], in_=o_sb[:])

    nc.compile()
    return nc


def _bf(a):
    return np.ascontiguousarray(a.astype(ml_dtypes.bfloat16))


def host_route(x_flat, gate_W, gate_b, temperature):
    """Replicates the reference router + expert-choice top-k in numpy."""
    logits = x_flat.astype(np.float32) @ gate_W + gate_b
    t = max(float(np.asarray(temperature).reshape(-1)[0]), 0.1)
    z = logits / t
    z = z - z.max(axis=1, keepdims=True)
    p = np.exp(z)
    p = p / p.sum(axis=1, keepdims=True)
    order = np.argsort(-p, axis=0, kind="stable")
    sel = order[:CAP]  # [CAP, 8]
    return p, sel


def _wk_layout(Wk):
    """[D, H] f32 -> [2, 128, KD, MH2, 128] bf16 (half-major over H)."""
    halves = []
    for h in range(2):
        w = Wk[:, h * (H // 2):(h + 1) * (H // 2)]
        halves.append(w.reshape(KD, 128, MH2, 128).transpose(1, 0, 2, 3))
    return _bf(np.stack(halves))


def _wv_layout(Wv):
    """[H, D] f32 -> [2, 128, MH2, ND2, 512] bf16 (half-major over H)."""
    halves = []
    for h in range(2):
        w = Wv[h * (H // 2):(h + 1) * (H // 2), :]
        halves.append(w.reshape(MH2, 128, ND2, 512).transpose(1, 0, 2, 3))
    return _bf(np.stack(halves))


def _shared_slices(c):
    """Token ranges of the flat [16384] space assigned to core c's B block
    (and core 7's A block)."""
    if c < 7:
        return (c * NTB, (c + 1) * NTB)
    return (7 * NTB, NTOK)  # core 7 covers the remainder across A+B


def prepare_in_maps(inputs):
    x = np.asarray(inputs["x"], np.float32).reshape(NTOK, D)
    p, sel = host_route(
        x, np.asarray(inputs["gate_W"], np.float32),
        np.asarray(inputs["gate_b"], np.float32),
        np.asarray(inputs["temperature"], np.float32),
    )

    sWk = np.asarray(inputs["sWk"], np.float32)
    sWv = np.asarray(inputs["sWv"], np.float32)
    sbk = np.asarray(inputs["sbk"], np.float32)
    Wk = np.asarray(inputs["Wk"], np.float32)
    Wv = np.asarray(inputs["Wv"], np.float32)
    bk = np.asarray(inputs["bk"], np.float32)

    swk_l = _wk_layout(sWk)
    swv_l = _wv_layout(sWv)
    sbk_l = np.ascontiguousarray(sbk.reshape(MH, 128).T)

    in_maps = []
    for c in range(NC):
        xtok = np.zeros((NT, D), np.float32)
        if c < 7:
            g = np.sort(sel[:, c])
            xtok[:CAP] = x[g]
            b0, b1 = _shared_slices(c)
            xtok[NTA:NTA + (b1 - b0)] = x[b0:b1]
            wks_c = np.stack([_wk_layout(Wk[c]), swk_l])
            wvs_c = np.stack([_wv_layout(Wv[c]), swv_l])
            bks_c = np.stack([np.ascontiguousarray(bk[c].reshape(MH, 128).T), sbk_l])
        else:
            b0, b1 = _shared_slices(c)  # 13440..16384
            nsh = b1 - b0  # 2944
            xtok[:NTA] = x[b0:b0 + NTA]
            xtok[NTA:NTA + (nsh - NTA)] = x[b0 + NTA:b1]
            wks_c = np.stack([swk_l, swk_l])
            wvs_c = np.stack([swv_l, swv_l])
            bks_c = np.stack([sbk_l, sbk_l])
        xT_c = np.ascontiguousarray(
            xtok.T.reshape(KD, 128, NT).transpose(1, 0, 2))
        in_maps.append({
            "xT": _bf(xT_c),
            "wks": wks_c, "wvs": wvs_c,
            "bks": np.ascontiguousarray(bks_c),
        })
    return in_maps, p, sel


_CACHED = None


def kernel(**inputs):
    global _CACHED
    if _CACHED is None:
        _CACHED = build_program()
    nc = _CACHED
    in_maps, p, sel = prepare_in_maps(inputs)
    res = run_bass_kernel_spmd(nc, in_maps, list(range(NC)))
    outs = [np.asarray(res.results[c]["out"], ml_dtypes.bfloat16)
            .astype(np.float32) for c in range(NC)]

    bv = np.asarray(inputs["bv"], np.float32)
    sbv = np.asarray(inputs["sbv"], np.float32)
    jump = np.asarray(inputs["jump"], np.float32)

    final = np.empty((NTOK, D), np.float32)
    # shared expert (+ sbv) for every token, from the owning core
    for c in range(7):
        b0, b1 = _shared_slices(c)
        final[b0:b1] = outs[c][NTA:NTA + (b1 - b0)]
    b0, b1 = _shared_slices(7)
    final[b0:b0 + NTA] = outs[7][:NTA]
    final[b0 + NTA:b1] = outs[7][NTA:NTA + (b1 - b0 - NTA)]
    final += sbv
    # FF experts: score-scaled, bv folded, scatter-added to owning tokens
    for c in range(7):
        g = np.sort(sel[:, c])
        final[g] += (outs[c][:CAP] + bv[c]) * p[g, c][:, None]
    # constant 'jump' expert
    m7 = sel[:, FF]
    final[m7] += jump[None, :] * p[m7, FF][:, None]
    return final.reshape(B, S, D)


if __name__ == "__main__":
    d = np.load("/root/problem/ref_inputs.npz")
    exp = np.load("/root/problem/ref_out.npy")
    got = kernel(**{k: d[k] for k in d.files})
    err = np.abs(got - exp)
    print("absmax rel:", err.max() / np.abs(exp).max())
    print("rms rel:", np.sqrt((err ** 2).mean()) / exp.std())


# revision 20
# speedup vs baseline: 1.9345x; 1.1032x over previous
"""Trainium2 Bass kernel for nn_ExpertsFeedForward (MoE expert-choice routing).

Sharding: expert-parallel with host-side token dispatch. The router
(softmax + expert-choice top-k) runs on host, as does the gather/scatter-add
"all-to-all" and all rank-1 epilogues (bv/sbv biases, jump expert, score
scaling). Each core runs two dense FFN blocks over pre-gathered tokens:

  block A (1664 tok): cores 0-6 -> that core's FF expert; core 7 -> shared
  block B (1920 tok): a 1920-token slice of the shared expert's tokens

so per-core compute is a balanced ~3.5k token-passes of gelu-FFN. All device
DMAs are large contiguous transfers of host-prelaid tensors (no gpsimd
gather/scatter, no transposing DMA), gelu runs on the Activation engine with
the bk bias fused, and the two GEMMs per chunk keep PE busy continuously.
"""

import numpy as np
import ml_dtypes

import concourse.bass as bass
import concourse.mybir as mybir
import concourse.bacc as bacc
import concourse.tile as tile
from concourse.bass_utils import run_bass_kernel_spmd

BF16 = mybir.dt.bfloat16
F32 = mybir.dt.float32

NC = 8            # cores
B, S = 8, 2048
D = 1024          # d_model
H = 4096          # d_ff
FF = 7            # matmul experts
E = 8             # router experts (7 FF + jump)
NTOK = 16384      # total tokens
CAP = 1638        # expert capacity
KD = D // 128     # 8 contraction tiles over d
MH = H // 128     # 32 tiles over h
MH2 = MH // 2     # 16 (half-split for DMA descriptor sizing)
MH4 = MH // 4     # 8  (quarter-split so GEMM1 starts after the first piece)
ND2 = 2           # 512-wide n chunks over D
NTA = 1664        # block-A layout stride (13*128 >= CAP; only CAP computed)
NTB = 1844        # block-B (shared) tokens per core (balanced: CAP+NTB==3482)
NT = NTA + NTB    # 3508 token slots per core
GELU = mybir.ActivationFunctionType.Gelu_apprx_tanh


CHW = 384  # token chunk width (must be a multiple of 128)


def _chunks(base, total):
    """Split [base, base+total) into CHW-token chunks (+mult-of-128 tail)."""
    out = []
    t = 0
    while t < total:
        cw = min(CHW, total - t)
        out.append((base + t, cw))
        t += cw
    return out


def build_program(reps=1):
    nc = bacc.Bacc("TRN2", target_bir_lowering=False, debug=False, num_devices=NC)

    # ---- per-core external inputs (host-prelaid, all contiguous) ----
    xT = nc.dram_tensor("xT", [128, KD, NT], BF16, kind="ExternalInput")
    wks = nc.dram_tensor("wks", [2, 4, 128, KD * MH4 * 128], BF16,
                         kind="ExternalInput")
    wvs = nc.dram_tensor("wvs", [2, 2, 128, MH2, ND2, 512], BF16, kind="ExternalInput")
    bks = nc.dram_tensor("bks", [2, 128, MH], F32, kind="ExternalInput")
    out = nc.dram_tensor("out", [NT, D], BF16, kind="ExternalOutput")

    with tile.TileContext(nc) as tc:
        with (
            tc.tile_pool(name="w", bufs=1) as wp,
            tc.tile_pool(name="io", bufs=2) as iop,
            tc.tile_pool(name="act", bufs=2) as ap_,
            tc.tile_pool(name="o", bufs=3) as op_,
            tc.tile_pool(name="ps1", bufs=2, space="PSUM") as pp1,
            tc.tile_pool(name="ps2", bufs=4, space="PSUM") as pp2,
        ):
            def issue_xc(t0, cw):
                xc = iop.tile([128, KD, CHW], BF16, tag="xc")
                nc.sync.dma_start(out=xc[:, :, :cw], in_=xT[:, :, t0:t0 + cw])
                return xc

            def issue_head(blk):
                """x chunk 0, then wk (quarter 0 first so GEMM1 can start as
                soon as it lands), then bias, then the remaining quarters."""
                t0, cw = _chunks(NTA, NTB)[0] if blk else _chunks(0, CAP)[0]
                xc0 = issue_xc(t0, cw)
                wk_sb = wp.tile([128, 4, KD * MH4 * 128], BF16, tag="wk")
                nc.sync.dma_start(out=wk_sb[:, 0], in_=wks[blk, 0])
                bk_sb = iop.tile([128, MH], F32, tag="bk")
                nc.sync.dma_start(out=bk_sb[:], in_=bks[blk])
                for q in range(1, 4):
                    nc.sync.dma_start(out=wk_sb[:, q], in_=wks[blk, q])
                return xc0, wk_sb, bk_sb

            for _rep in range(reps):
                heads = {0: issue_head(0)}
                for blk in range(2):
                    # block A computes only the CAP real tokens (tail chunk 102)
                    chunks = _chunks(0 if blk == 0 else NTA,
                                     CAP if blk == 0 else NTB)
                    xc0, wk_sb, bk_sb = heads.pop(blk)
                    xcs = {0: xc0, 1: issue_xc(*chunks[1])}
                    # wv after the x/wk head so it can't jump the DMA queue
                    wv_sb = wp.tile([128, 2, MH2, ND2, 512], BF16, tag="wv")
                    for h in range(2):
                        nc.sync.dma_start(out=wv_sb[:, h], in_=wvs[blk, h])

                    for ci, (t0, cw) in enumerate(chunks):
                        xc = xcs.pop(ci, None)
                        if xc is None:
                            xc = issue_xc(t0, cw)
                        hT = ap_.tile([128, MH, CHW], BF16, tag="hT")
                        for m in range(MH):
                            ps1 = pp1.tile([128, CHW], F32, tag="ps1")
                            for k in range(KD):
                                st = (k * MH4 + m % MH4) * 128
                                nc.tensor.matmul(
                                    ps1[:, :cw], wk_sb[:, m // MH4, st:st + 128],
                                    xc[:, k, :cw], start=(k == 0), stop=(k == KD - 1))
                            nc.scalar.activation(hT[:, m, :cw], ps1[:, :cw], GELU,
                                                 bias=bk_sb[:, m:m + 1])
                        if blk == 0 and ci == len(chunks) - 1:
                            # next block's head streams in during this block's
                            # tail GEMM2 (wk WAR clears at the G1 just issued)
                            heads[1] = issue_head(1)
                        for tt in range((cw + 127) // 128):
                            tw = min(128, cw - tt * 128)
                            o_sb = op_.tile([128, D], BF16, tag="o")
                            for n in range(ND2):
                                ps2 = pp2.tile([128, 512], F32, tag="ps2")
                                for m in range(MH):
                                    nc.tensor.matmul(
                                        ps2[:tw], hT[:, m, tt * 128:tt * 128 + tw],
                                        wv_sb[:, m // MH2, m % MH2, n, :],
                                        start=(m == 0), stop=(m == MH - 1))
                                nc.vector.tensor_copy(o_sb[:tw, n * 512:(n + 1) * 512],
                                                      ps2[:tw])
                            g0 = t0 + tt * 128
                            nc.sync.dma_start(out=out[g0:g0 + tw, :], in_=o_sb[:tw])

    nc.compile()
    return nc


def _bf(a):
    return np.ascontiguousarray(a.astype(ml_dtypes.bfloat16))


def host_route(x_flat, gate_W, gate_b, temperature):
    """Replicates the reference router + expert-choice top-k in numpy."""
    logits = x_flat.astype(np.float32) @ gate_W + gate_b
    t = max(float(np.asarray(temperature).reshape(-1)[0]), 0.1)
    z = logits / t
    z = z - z.max(axis=1, keepdims=True)
    p = np.exp(z)
    p = p / p.sum(axis=1, keepdims=True)
    order = np.argsort(-p, axis=0, kind="stable")
    sel = order[:CAP]  # [CAP, 8]
    return p, sel


def _wk_layout(Wk):
    """[D, H] f32 -> [4, 128, KD*MH4*128] bf16 (quarter-major over H)."""
    quarters = []
    for q in range(4):
        w = Wk[:, q * (H // 4):(q + 1) * (H // 4)]
        quarters.append(w.reshape(KD, 128, MH4, 128).transpose(1, 0, 2, 3)
                        .reshape(128, KD * MH4 * 128))
    return _bf(np.stack(quarters))


def _wv_layout(Wv):
    """[H, D] f32 -> [2, 128, MH2, ND2, 512] bf16 (half-major over H)."""
    halves = []
    for h in range(2):
        w = Wv[h * (H // 2):(h + 1) * (H // 2), :]
        halves.append(w.reshape(MH2, 128, ND2, 512).transpose(1, 0, 2, 3))
    return _bf(np.stack(halves))


def _shared_slices(c):
    """Token ranges of the flat [16384] space assigned to core c's B block
    (core 7 also covers [b0, b0+CAP) in its A block)."""
    if c < 7:
        return (c * NTB, (c + 1) * NTB)
    return (7 * NTB, NTOK)  # core 7: first CAP in block A, rest in block B


def prepare_in_maps(inputs):
    x = np.asarray(inputs["x"], np.float32).reshape(NTOK, D)
    p, sel = host_route(
        x, np.asarray(inputs["gate_W"], np.float32),
        np.asarray(inputs["gate_b"], np.float32),
        np.asarray(inputs["temperature"], np.float32),
    )

    sWk = np.asarray(inputs["sWk"], np.float32)
    sWv = np.asarray(inputs["sWv"], np.float32)
    sbk = np.asarray(inputs["sbk"], np.float32)
    Wk = np.asarray(inputs["Wk"], np.float32)
    Wv = np.asarray(inputs["Wv"], np.float32)
    bk = np.asarray(inputs["bk"], np.float32)

    swk_l = _wk_layout(sWk)
    swv_l = _wv_layout(sWv)
    sbk_l = np.ascontiguousarray(sbk.reshape(MH, 128).T)

    in_maps = []
    for c in range(NC):
        xtok = np.zeros((NT, D), np.float32)
        if c < 7:
            g = np.sort(sel[:, c])
            xtok[:CAP] = x[g]
            b0, b1 = _shared_slices(c)
            xtok[NTA:NTA + (b1 - b0)] = x[b0:b1]
            wks_c = np.stack([_wk_layout(Wk[c]), swk_l])
            wvs_c = np.stack([_wv_layout(Wv[c]), swv_l])
            bks_c = np.stack([np.ascontiguousarray(bk[c].reshape(MH, 128).T), sbk_l])
        else:
            b0, b1 = _shared_slices(c)  # 12908..16384
            xtok[:CAP] = x[b0:b0 + CAP]
            xtok[NTA:NTA + (b1 - b0 - CAP)] = x[b0 + CAP:b1]
            wks_c = np.stack([swk_l, swk_l])
            wvs_c = np.stack([swv_l, swv_l])
            bks_c = np.stack([sbk_l, sbk_l])
        xT_c = np.ascontiguousarray(
            xtok.T.reshape(KD, 128, NT).transpose(1, 0, 2))
        in_maps.append({
            "xT": _bf(xT_c),
            "wks": wks_c, "wvs": wvs_c,
            "bks": np.ascontiguousarray(bks_c),
        })
    return in_maps, p, sel


_CACHED = None


def kernel(**inputs):
    global _CACHED
    if _CACHED is None:
        _CACHED = build_program()
    nc = _CACHED
    in_maps, p, sel = prepare_in_maps(inputs)
    res = run_bass_kernel_spmd(nc, in_maps, list(range(NC)))
    outs = [np.asarray(res.results[c]["out"], ml_dtypes.bfloat16)
            .astype(np.float32) for c in range(NC)]

    bv = np.asarray(inputs["bv"], np.float32)
    sbv = np.asarray(inputs["sbv"], np.float32)
    jump = np.asarray(inputs["jump"], np.float32)

    final = np.empty((NTOK, D), np.float32)
    # shared expert (+ sbv) for every token, from the owning core
    for c in range(7):
        b0, b1 = _shared_slices(c)
        final[b0:b1] = outs[c][NTA:NTA + (b1 - b0)]
    b0, b1 = _shared_slices(7)
    final[b0:b0 + CAP] = outs[7][:CAP]
    final[b0 + CAP:b1] = outs[7][NTA:NTA + (b1 - b0 - CAP)]
    final += sbv
    # FF experts: score-scaled, bv folded, scatter-added to owning tokens
    for c in range(7):
        g = np.sort(sel[:, c])
        final[g] += (outs[c][:CAP] + bv[c]) * p[g, c][:, None]
    # constant 'jump' expert
    m7 = sel[:, FF]
    final[m7] += jump[None, :] * p[m7, FF][:, None]
    return final.reshape(B, S, D)


if __name__ == "__main__":
    d = np.load("/root/problem/ref_inputs.npz")
    exp = np.load("/root/problem/ref_out.npy")
    got = kernel(**{k: d[k] for k in d.files})
    err = np.abs(got - exp)
    print("absmax rel:", err.max() / np.abs(exp).max())
    print("rms rel:", np.sqrt((err ** 2).mean()) / exp.std())


# revision 23
# speedup vs baseline: 1.9420x; 1.0039x over previous
"""Trainium2 Bass kernel for nn_ExpertsFeedForward (MoE expert-choice routing).

Sharding: expert-parallel with host-side token dispatch. The router
(softmax + expert-choice top-k) runs on host, as does the gather/scatter-add
"all-to-all" and all rank-1 epilogues (bv/sbv biases, jump expert, score
scaling). Each core runs two dense FFN blocks over pre-gathered tokens:

  block A (1664 tok): cores 0-6 -> that core's FF expert; core 7 -> shared
  block B (1920 tok): a 1920-token slice of the shared expert's tokens

so per-core compute is a balanced ~3.5k token-passes of gelu-FFN. All device
DMAs are large contiguous transfers of host-prelaid tensors (no gpsimd
gather/scatter, no transposing DMA), gelu runs on the Activation engine with
the bk bias fused, and the two GEMMs per chunk keep PE busy continuously.
"""

import numpy as np
import ml_dtypes

import concourse.bass as bass
import concourse.mybir as mybir
import concourse.bacc as bacc
import concourse.tile as tile
from concourse.bass_utils import run_bass_kernel_spmd

BF16 = mybir.dt.bfloat16
F32 = mybir.dt.float32

NC = 8            # cores
B, S = 8, 2048
D = 1024          # d_model
H = 4096          # d_ff
FF = 7            # matmul experts
E = 8             # router experts (7 FF + jump)
NTOK = 16384      # total tokens
CAP = 1638        # expert capacity
KD = D // 128     # 8 contraction tiles over d
MH = H // 128     # 32 tiles over h
MH2 = MH // 2     # 16 (half-split for DMA descriptor sizing)
MH4 = MH // 4     # 8  (quarter-split so GEMM1 starts after the first piece)
ND2 = 2           # 512-wide n chunks over D
NTA = 1664        # block-A layout stride (13*128 >= CAP; only CAP computed)
NTB = 1844        # block-B (shared) tokens per core (balanced: CAP+NTB==3482)
NT = NTA + NTB    # 3508 token slots per core
GELU = mybir.ActivationFunctionType.Gelu_apprx_tanh


CHW = 384  # token chunk width (must be a multiple of 128)


def _chunks(base, total):
    """Split [base, base+total) into CHW-token chunks (+mult-of-128 tail)."""
    out = []
    t = 0
    while t < total:
        cw = min(CHW, total - t)
        out.append((base + t, cw))
        t += cw
    return out


def build_program(reps=1):
    nc = bacc.Bacc("TRN2", target_bir_lowering=False, debug=False, num_devices=NC)

    # ---- per-core external inputs (host-prelaid, all contiguous) ----
    xT = nc.dram_tensor("xT", [128, KD, NT], BF16, kind="ExternalInput")
    wks = nc.dram_tensor("wks", [2, 4, 128, KD * MH4 * 128], BF16,
                         kind="ExternalInput")
    wvs = nc.dram_tensor("wvs", [2, 2, 128, MH2, ND2, 512], BF16, kind="ExternalInput")
    bks = nc.dram_tensor("bks", [2, 128, MH], F32, kind="ExternalInput")
    out = nc.dram_tensor("out", [NT, D], BF16, kind="ExternalOutput")

    with tile.TileContext(nc) as tc:
        with (
            tc.tile_pool(name="w", bufs=1) as wp,
            tc.tile_pool(name="io", bufs=2) as iop,
            tc.tile_pool(name="act", bufs=2) as ap_,
            tc.tile_pool(name="o", bufs=3) as op_,
            tc.tile_pool(name="ps1", bufs=2, space="PSUM") as pp1,
            tc.tile_pool(name="ps2", bufs=4, space="PSUM") as pp2,
        ):
            def issue_xc(t0, cw):
                xc = iop.tile([128, KD, CHW], BF16, tag="xc")
                nc.sync.dma_start(out=xc[:, :, :cw], in_=xT[:, :, t0:t0 + cw])
                return xc

            def issue_head(blk):
                """x chunk 0, then wk (quarter 0 first so GEMM1 can start as
                soon as it lands), then bias, then the remaining quarters."""
                t0, cw = _chunks(NTA, NTB)[0] if blk else _chunks(0, CAP)[0]
                xc0 = issue_xc(t0, cw)
                wk_sb = wp.tile([128, 4, KD * MH4 * 128], BF16, tag="wk")
                half = KD * MH4 * 128 // 2  # m-major: first piece covers mq 0-3
                nc.sync.dma_start(out=wk_sb[:, 0, :half], in_=wks[blk, 0, :, :half])
                bk_sb = iop.tile([128, MH], F32, tag="bk")
                nc.sync.dma_start(out=bk_sb[:], in_=bks[blk])
                nc.sync.dma_start(out=wk_sb[:, 0, half:], in_=wks[blk, 0, :, half:])
                for q in range(1, 4):
                    nc.sync.dma_start(out=wk_sb[:, q], in_=wks[blk, q])
                return xc0, wk_sb, bk_sb

            for _rep in range(reps):
                heads = {0: issue_head(0)}
                for blk in range(2):
                    # block A computes only the CAP real tokens (tail chunk 102)
                    chunks = _chunks(0 if blk == 0 else NTA,
                                     CAP if blk == 0 else NTB)
                    xc0, wk_sb, bk_sb = heads.pop(blk)
                    xcs = {0: xc0, 1: issue_xc(*chunks[1])}
                    # wv after the x/wk head so it can't jump the DMA queue
                    wv_sb = wp.tile([128, 2, MH2, ND2, 512], BF16, tag="wv")
                    for h in range(2):
                        nc.sync.dma_start(out=wv_sb[:, h], in_=wvs[blk, h])

                    for ci, (t0, cw) in enumerate(chunks):
                        xc = xcs.pop(ci, None)
                        if xc is None:
                            xc = issue_xc(t0, cw)
                        hT = ap_.tile([128, MH, CHW], BF16, tag="hT")
                        for m in range(MH):
                            ps1 = pp1.tile([128, CHW], F32, tag="ps1")
                            for k in range(KD):
                                st = ((m % MH4) * KD + k) * 128
                                nc.tensor.matmul(
                                    ps1[:, :cw], wk_sb[:, m // MH4, st:st + 128],
                                    xc[:, k, :cw], start=(k == 0), stop=(k == KD - 1))
                            nc.scalar.activation(hT[:, m, :cw], ps1[:, :cw], GELU,
                                                 bias=bk_sb[:, m:m + 1])
                        if blk == 0 and ci == len(chunks) - 1:
                            # next block's head streams in during this block's
                            # tail GEMM2 (wk WAR clears at the G1 just issued)
                            heads[1] = issue_head(1)
                        for tt in range((cw + 127) // 128):
                            tw = min(128, cw - tt * 128)
                            o_sb = op_.tile([128, D], BF16, tag="o")
                            for n in range(ND2):
                                ps2 = pp2.tile([128, 512], F32, tag="ps2")
                                for m in range(MH):
                                    nc.tensor.matmul(
                                        ps2[:tw], hT[:, m, tt * 128:tt * 128 + tw],
                                        wv_sb[:, m // MH2, m % MH2, n, :],
                                        start=(m == 0), stop=(m == MH - 1))
                                nc.vector.tensor_copy(o_sb[:tw, n * 512:(n + 1) * 512],
                                                      ps2[:tw])
                            g0 = t0 + tt * 128
                            nc.sync.dma_start(out=out[g0:g0 + tw, :], in_=o_sb[:tw])

    nc.compile()
    return nc


def _bf(a):
    return np.ascontiguousarray(a.astype(ml_dtypes.bfloat16))


def host_route(x_flat, gate_W, gate_b, temperature):
    """Replicates the reference router + expert-choice top-k in numpy."""
    logits = x_flat.astype(np.float32) @ gate_W + gate_b
    t = max(float(np.asarray(temperature).reshape(-1)[0]), 0.1)
    z = logits / t
    z = z - z.max(axis=1, keepdims=True)
    p = np.exp(z)
    p = p / p.sum(axis=1, keepdims=True)
    order = np.argsort(-p, axis=0, kind="stable")
    sel = order[:CAP]  # [CAP, 8]
    return p, sel


def _wk_layout(Wk):
    """[D, H] f32 -> [4, 128, MH4*KD*128] bf16, quarter-major over H with
    m-major interior so the first half-quarter already covers m-tiles 0-3."""
    quarters = []
    for q in range(4):
        w = Wk[:, q * (H // 4):(q + 1) * (H // 4)]
        quarters.append(w.reshape(KD, 128, MH4, 128).transpose(1, 2, 0, 3)
                        .reshape(128, MH4 * KD * 128))
    return _bf(np.stack(quarters))


def _wv_layout(Wv):
    """[H, D] f32 -> [2, 128, MH2, ND2, 512] bf16 (half-major over H)."""
    halves = []
    for h in range(2):
        w = Wv[h * (H // 2):(h + 1) * (H // 2), :]
        halves.append(w.reshape(MH2, 128, ND2, 512).transpose(1, 0, 2, 3))
    return _bf(np.stack(halves))


def _shared_slices(c):
    """Token ranges of the flat [16384] space assigned to core c's B block
    (core 7 also covers [b0, b0+CAP) in its A block)."""
    if c < 7:
        return (c * NTB, (c + 1) * NTB)
    return (7 * NTB, NTOK)  # core 7: first CAP in block A, rest in block B


def prepare_in_maps(inputs):
    x = np.asarray(inputs["x"], np.float32).reshape(NTOK, D)
    p, sel = host_route(
        x, np.asarray(inputs["gate_W"], np.float32),
        np.asarray(inputs["gate_b"], np.float32),
        np.asarray(inputs["temperature"], np.float32),
    )

    sWk = np.asarray(inputs["sWk"], np.float32)
    sWv = np.asarray(inputs["sWv"], np.float32)
    sbk = np.asarray(inputs["sbk"], np.float32)
    Wk = np.asarray(inputs["Wk"], np.float32)
    Wv = np.asarray(inputs["Wv"], np.float32)
    bk = np.asarray(inputs["bk"], np.float32)

    swk_l = _wk_layout(sWk)
    swv_l = _wv_layout(sWv)
    sbk_l = np.ascontiguousarray(sbk.reshape(MH, 128).T)

    in_maps = []
    for c in range(NC):
        xtok = np.zeros((NT, D), np.float32)
        if c < 7:
            g = np.sort(sel[:, c])
            xtok[:CAP] = x[g]
            b0, b1 = _shared_slices(c)
            xtok[NTA:NTA + (b1 - b0)] = x[b0:b1]
            wks_c = np.stack([_wk_layout(Wk[c]), swk_l])
            wvs_c = np.stack([_wv_layout(Wv[c]), swv_l])
            bks_c = np.stack([np.ascontiguousarray(bk[c].reshape(MH, 128).T), sbk_l])
        else:
            b0, b1 = _shared_slices(c)  # 12908..16384
            xtok[:CAP] = x[b0:b0 + CAP]
            xtok[NTA:NTA + (b1 - b0 - CAP)] = x[b0 + CAP:b1]
            wks_c = np.stack([swk_l, swk_l])
            wvs_c = np.stack([swv_l, swv_l])
            bks_c = np.stack([sbk_l, sbk_l])
        xT_c = np.ascontiguousarray(
            xtok.T.reshape(KD, 128, NT).transpose(1, 0, 2))
        in_maps.append({
            "xT": _bf(xT_c),
            "wks": wks_c, "wvs": wvs_c,
            "bks": np.ascontiguousarray(bks_c),
        })
    return in_maps, p, sel


_CACHED = None


def kernel(**inputs):
    global _CACHED
    if _CACHED is None:
        _CACHED = build_program()
    nc = _CACHED
    in_maps, p, sel = prepare_in_maps(inputs)
    res = run_bass_kernel_spmd(nc, in_maps, list(range(NC)))
    outs = [np.asarray(res.results[c]["out"], ml_dtypes.bfloat16)
            .astype(np.float32) for c in range(NC)]

    bv = np.asarray(inputs["bv"], np.float32)
    sbv = np.asarray(inputs["sbv"], np.float32)
    jump = np.asarray(inputs["jump"], np.float32)

    final = np.empty((NTOK, D), np.float32)
    # shared expert (+ sbv) for every token, from the owning core
    for c in range(7):
        b0, b1 = _shared_slices(c)
        final[b0:b1] = outs[c][NTA:NTA + (b1 - b0)]
    b0, b1 = _shared_slices(7)
    final[b0:b0 + CAP] = outs[7][:CAP]
    final[b0 + CAP:b1] = outs[7][NTA:NTA + (b1 - b0 - CAP)]
    final += sbv
    # FF experts: score-scaled, bv folded, scatter-added to owning tokens
    for c in range(7):
        g = np.sort(sel[:, c])
        final[g] += (outs[c][:CAP] + bv[c]) * p[g, c][:, None]
    # constant 'jump' expert
    m7 = sel[:, FF]
    final[m7] += jump[None, :] * p[m7, FF][:, None]
    return final.reshape(B, S, D)


if __name__ == "__main__":
    d = np.load("/root/problem/ref_inputs.npz")
    exp = np.load("/root/problem/ref_out.npy")
    got = kernel(**{k: d[k] for k in d.files})
    err = np.abs(got - exp)
    print("absmax rel:", err.max() / np.abs(exp).max())
    print("rms rel:", np.sqrt((err ** 2).mean()) / exp.std())


# revision 24
# speedup vs baseline: 1.9532x; 1.0058x over previous
"""Trainium2 Bass kernel for nn_ExpertsFeedForward (MoE expert-choice routing).

Sharding: expert-parallel with host-side token dispatch. The router
(softmax + expert-choice top-k) runs on host, as does the gather/scatter-add
"all-to-all" and all rank-1 epilogues (bv/sbv biases, jump expert, score
scaling). Each core runs two dense FFN blocks over pre-gathered tokens:

  block A (1664 tok): cores 0-6 -> that core's FF expert; core 7 -> shared
  block B (1920 tok): a 1920-token slice of the shared expert's tokens

so per-core compute is a balanced ~3.5k token-passes of gelu-FFN. All device
DMAs are large contiguous transfers of host-prelaid tensors (no gpsimd
gather/scatter, no transposing DMA), gelu runs on the Activation engine with
the bk bias fused, and the two GEMMs per chunk keep PE busy continuously.
"""

import numpy as np
import ml_dtypes

import concourse.bass as bass
import concourse.mybir as mybir
import concourse.bacc as bacc
import concourse.tile as tile
from concourse.bass_utils import run_bass_kernel_spmd

BF16 = mybir.dt.bfloat16
F32 = mybir.dt.float32

NC = 8            # cores
B, S = 8, 2048
D = 1024          # d_model
H = 4096          # d_ff
FF = 7            # matmul experts
E = 8             # router experts (7 FF + jump)
NTOK = 16384      # total tokens
CAP = 1638        # expert capacity
KD = D // 128     # 8 contraction tiles over d
MH = H // 128     # 32 tiles over h
MH2 = MH // 2     # 16 (half-split for DMA descriptor sizing)
MH4 = MH // 4     # 8  (quarter-split so GEMM1 starts after the first piece)
ND2 = 2           # 512-wide n chunks over D
NTA = 1664        # block-A layout stride (13*128 >= CAP; only CAP computed)
NTB = 1844        # block-B (shared) tokens per core (balanced: CAP+NTB==3482)
NT = NTA + NTB    # 3508 token slots per core
GELU = mybir.ActivationFunctionType.Gelu_apprx_tanh


CHW = 256  # token chunk width (also GEMM2 psum width; 1KB psum tiles)


def _chunks(base, total):
    """Split [base, base+total) into CHW-token chunks (+mult-of-128 tail)."""
    out = []
    t = 0
    while t < total:
        cw = min(CHW, total - t)
        out.append((base + t, cw))
        t += cw
    return out


def build_program(reps=1):
    nc = bacc.Bacc("TRN2", target_bir_lowering=False, debug=False, num_devices=NC)

    # ---- per-core external inputs (host-prelaid, all contiguous) ----
    xT = nc.dram_tensor("xT", [128, KD, NT], BF16, kind="ExternalInput")
    wks = nc.dram_tensor("wks", [2, 4, 128, KD * MH4 * 128], BF16,
                         kind="ExternalInput")
    wvs = nc.dram_tensor("wvs", [2, 2, 128, MH2 * KD * 128], BF16,
                         kind="ExternalInput")
    bks = nc.dram_tensor("bks", [2, 128, MH], F32, kind="ExternalInput")
    out = nc.dram_tensor("out", [128, KD, NT], BF16, kind="ExternalOutput")

    with tile.TileContext(nc) as tc:
        with (
            tc.tile_pool(name="w", bufs=1) as wp,
            tc.tile_pool(name="io", bufs=2) as iop,
            tc.tile_pool(name="act", bufs=2) as ap_,
            tc.tile_pool(name="o", bufs=3) as op_,
            tc.tile_pool(name="ps1", bufs=2, space="PSUM") as pp1,
            tc.tile_pool(name="ps2", bufs=4, space="PSUM") as pp2,
        ):
            def issue_xc(t0, cw):
                xc = iop.tile([128, KD, CHW], BF16, tag="xc")
                nc.sync.dma_start(out=xc[:, :, :cw], in_=xT[:, :, t0:t0 + cw])
                return xc

            def issue_head(blk):
                """x chunk 0, then wk (quarter 0 first so GEMM1 can start as
                soon as it lands), then bias, then the remaining quarters."""
                t0, cw = _chunks(NTA, NTB)[0] if blk else _chunks(0, CAP)[0]
                xc0 = issue_xc(t0, cw)
                wk_sb = wp.tile([128, 4, KD * MH4 * 128], BF16, tag="wk")
                half = KD * MH4 * 128 // 2  # m-major: first piece covers mq 0-3
                nc.sync.dma_start(out=wk_sb[:, 0, :half], in_=wks[blk, 0, :, :half])
                bk_sb = iop.tile([128, MH], F32, tag="bk")
                nc.sync.dma_start(out=bk_sb[:], in_=bks[blk])
                nc.sync.dma_start(out=wk_sb[:, 0, half:], in_=wks[blk, 0, :, half:])
                for q in range(1, 4):
                    nc.sync.dma_start(out=wk_sb[:, q], in_=wks[blk, q])
                return xc0, wk_sb, bk_sb

            for _rep in range(reps):
                heads = {0: issue_head(0)}
                for blk in range(2):
                    # block A computes only the CAP real tokens (tail chunk 102)
                    chunks = _chunks(0 if blk == 0 else NTA,
                                     CAP if blk == 0 else NTB)
                    xc0, wk_sb, bk_sb = heads.pop(blk)
                    xcs = {0: xc0, 1: issue_xc(*chunks[1])}
                    # wv after the x/wk head so it can't jump the DMA queue
                    wv_sb = wp.tile([128, 2, MH2 * KD * 128], BF16, tag="wv")
                    for h in range(2):
                        nc.sync.dma_start(out=wv_sb[:, h], in_=wvs[blk, h])

                    for ci, (t0, cw) in enumerate(chunks):
                        xc = xcs.pop(ci, None)
                        if xc is None:
                            xc = issue_xc(t0, cw)
                        hT = ap_.tile([128, MH, CHW], BF16, tag="hT")
                        for m in range(MH):
                            ps1 = pp1.tile([128, CHW], F32, tag="ps1")
                            for k in range(KD):
                                st = ((m % MH4) * KD + k) * 128
                                nc.tensor.matmul(
                                    ps1[:, :cw], wk_sb[:, m // MH4, st:st + 128],
                                    xc[:, k, :cw], start=(k == 0), stop=(k == KD - 1))
                            nc.scalar.activation(hT[:, m, :cw], ps1[:, :cw], GELU,
                                                 bias=bk_sb[:, m:m + 1])
                        if blk == 0 and ci == len(chunks) - 1:
                            # next block's head streams in during this block's
                            # tail GEMM2 (wk WAR clears at the G1 just issued)
                            heads[1] = issue_head(1)
                        oT = op_.tile([128, KD, CHW], BF16, tag="o")
                        for nd in range(KD):
                            ps2 = pp2.tile([128, CHW], F32, tag="ps2")
                            for m in range(MH):
                                off = ((m % MH2) * KD + nd) * 128
                                nc.tensor.matmul(
                                    ps2[:, :cw], wv_sb[:, m // MH2, off:off + 128],
                                    hT[:, m, :cw], start=(m == 0), stop=(m == MH - 1))
                            nc.vector.tensor_copy(oT[:, nd, :cw], ps2[:, :cw])
                        nc.sync.dma_start(out=out[:, :, t0:t0 + cw], in_=oT[:, :, :cw])

    nc.compile()
    return nc


def _bf(a):
    return np.ascontiguousarray(a.astype(ml_dtypes.bfloat16))


def host_route(x_flat, gate_W, gate_b, temperature):
    """Replicates the reference router + expert-choice top-k in numpy."""
    logits = x_flat.astype(np.float32) @ gate_W + gate_b
    t = max(float(np.asarray(temperature).reshape(-1)[0]), 0.1)
    z = logits / t
    z = z - z.max(axis=1, keepdims=True)
    p = np.exp(z)
    p = p / p.sum(axis=1, keepdims=True)
    order = np.argsort(-p, axis=0, kind="stable")
    sel = order[:CAP]  # [CAP, 8]
    return p, sel


def _wk_layout(Wk):
    """[D, H] f32 -> [4, 128, MH4*KD*128] bf16, quarter-major over H with
    m-major interior so the first half-quarter already covers m-tiles 0-3."""
    quarters = []
    for q in range(4):
        w = Wk[:, q * (H // 4):(q + 1) * (H // 4)]
        quarters.append(w.reshape(KD, 128, MH4, 128).transpose(1, 2, 0, 3)
                        .reshape(128, MH4 * KD * 128))
    return _bf(np.stack(quarters))


def _wv_layout(Wv):
    """[H, D] f32 -> [2, 128, MH2*KD*128] bf16 (half-major over H; interior
    m-major then d-tile so GEMM2's lhsT is a flat 128-col slice)."""
    halves = []
    for h in range(2):
        w = Wv[h * (H // 2):(h + 1) * (H // 2), :]
        halves.append(w.reshape(MH2, 128, KD, 128).transpose(1, 0, 2, 3)
                      .reshape(128, MH2 * KD * 128))
    return _bf(np.stack(halves))


def _shared_slices(c):
    """Token ranges of the flat [16384] space assigned to core c's B block
    (core 7 also covers [b0, b0+CAP) in its A block)."""
    if c < 7:
        return (c * NTB, (c + 1) * NTB)
    return (7 * NTB, NTOK)  # core 7: first CAP in block A, rest in block B


def prepare_in_maps(inputs):
    x = np.asarray(inputs["x"], np.float32).reshape(NTOK, D)
    p, sel = host_route(
        x, np.asarray(inputs["gate_W"], np.float32),
        np.asarray(inputs["gate_b"], np.float32),
        np.asarray(inputs["temperature"], np.float32),
    )

    sWk = np.asarray(inputs["sWk"], np.float32)
    sWv = np.asarray(inputs["sWv"], np.float32)
    sbk = np.asarray(inputs["sbk"], np.float32)
    Wk = np.asarray(inputs["Wk"], np.float32)
    Wv = np.asarray(inputs["Wv"], np.float32)
    bk = np.asarray(inputs["bk"], np.float32)

    swk_l = _wk_layout(sWk)
    swv_l = _wv_layout(sWv)
    sbk_l = np.ascontiguousarray(sbk.reshape(MH, 128).T)

    in_maps = []
    for c in range(NC):
        xtok = np.zeros((NT, D), np.float32)
        if c < 7:
            g = np.sort(sel[:, c])
            xtok[:CAP] = x[g]
            b0, b1 = _shared_slices(c)
            xtok[NTA:NTA + (b1 - b0)] = x[b0:b1]
            wks_c = np.stack([_wk_layout(Wk[c]), swk_l])
            wvs_c = np.stack([_wv_layout(Wv[c]), swv_l])
            bks_c = np.stack([np.ascontiguousarray(bk[c].reshape(MH, 128).T), sbk_l])
        else:
            b0, b1 = _shared_slices(c)  # 12908..16384
            xtok[:CAP] = x[b0:b0 + CAP]
            xtok[NTA:NTA + (b1 - b0 - CAP)] = x[b0 + CAP:b1]
            wks_c = np.stack([swk_l, swk_l])
            wvs_c = np.stack([swv_l, swv_l])
            bks_c = np.stack([sbk_l, sbk_l])
        xT_c = np.ascontiguousarray(
            xtok.T.reshape(KD, 128, NT).transpose(1, 0, 2))
        in_maps.append({
            "xT": _bf(xT_c),
            "wks": wks_c, "wvs": wvs_c,
            "bks": np.ascontiguousarray(bks_c),
        })
    return in_maps, p, sel


_CACHED = None


def kernel(**inputs):
    global _CACHED
    if _CACHED is None:
        _CACHED = build_program()
    nc = _CACHED
    in_maps, p, sel = prepare_in_maps(inputs)
    res = run_bass_kernel_spmd(nc, in_maps, list(range(NC)))
    outs = [np.asarray(res.results[c]["out"], ml_dtypes.bfloat16)
            .astype(np.float32).transpose(2, 1, 0).reshape(NT, D)
            for c in range(NC)]

    bv = np.asarray(inputs["bv"], np.float32)
    sbv = np.asarray(inputs["sbv"], np.float32)
    jump = np.asarray(inputs["jump"], np.float32)

    final = np.empty((NTOK, D), np.float32)
    # shared expert (+ sbv) for every token, from the owning core
    for c in range(7):
        b0, b1 = _shared_slices(c)
        final[b0:b1] = outs[c][NTA:NTA + (b1 - b0)]
    b0, b1 = _shared_slices(7)
    final[b0:b0 + CAP] = outs[7][:CAP]
    final[b0 + CAP:b1] = outs[7][NTA:NTA + (b1 - b0 - CAP)]
    final += sbv
    # FF experts: score-scaled, bv folded, scatter-added to owning tokens
    for c in range(7):
        g = np.sort(sel[:, c])
        final[g] += (outs[c][:CAP] + bv[c]) * p[g, c][:, None]
    # constant 'jump' expert
    m7 = sel[:, FF]
    final[m7] += jump[None, :] * p[m7, FF][:, None]
    return final.reshape(B, S, D)


if __name__ == "__main__":
    d = np.load("/root/problem/ref_inputs.npz")
    exp = np.load("/root/problem/ref_out.npy")
    got = kernel(**{k: d[k] for k in d.files})
    err = np.abs(got - exp)
    print("absmax rel:", err.max() / np.abs(exp).max())
    print("rms rel:", np.sqrt((err ** 2).mean()) / exp.std())
